# revision 16
# baseline (speedup 1.0000x reference)
"""BitLinear (8-bit abs-max act / mean-abs weight quant) tensor-parallel kernel
for 8 Trainium2 NeuronCores — v3: startup-overlapped schedule.

Math (matches the reference exactly):
    gamma = max(max|x|, 1e-5)                    (per-tensor scalar)
    xq    = clip(round(x * (128/gamma)), -128, 127)
    beta  = max(mean|w|, 1e-5)                   (per-tensor scalar)
    wq    = clip(round(|w|/beta), -1, 1)  == (|w| > beta/2) in {0,1}
    y     = (xq @ wq.T) * (beta*gamma/128)

Sharding: weight rows (out_features) split across 8 cores; activations
replicated; per-core scalar partials combined with two tiny AllReduces
(add for the |w| sum, max for gamma).  The GEMM runs in bf16 which is
exact here (xq in [-128,127], wq in {0,1}, fp32 PSUM accumulation).

Schedule notes (v3, from trace evidence):
  - The sync-engine (SP) hardware DMA queue moves ~0.31us/8KB packet;
    the Activation-engine queue is ~2.6x slower.  So all bulk traffic
    (wt stats, wq re-read, x loads, y writes, steady-state transposes)
    rides the sync queue; only the gamma stat slice + tiny scalar
    round-trips + the first four transposes use the scalar queue.
  - The first collective pays ~90-130us of one-time CC warmup; later
    ones cost ~26us.  A dependency-free dummy AllReduce is issued at
    t~8us to prepay the warmup, then beta (critical) and gamma.
  - wt stats are accumulated on the scalar engine (activation Abs with
    accum_out) so the vector engine is free to chase the wq quantize;
    x quantization runs on the otherwise idle gpsimd engine.
  - The wq re-read DMAs are issued eagerly (only the quantize op waits
    for beta); the first two token blocks run interleaved per-j in two
    PSUM groups, consuming wq slabs as they are produced.
"""

import sys

import numpy as np

if "/opt/trn_rl_repo" not in sys.path:
    sys.path.insert(0, "/opt/trn_rl_repo")

import concourse.bass as bass
import concourse.mybir as mybir
import concourse.tile as tile
from concourse.bass_utils import run_bass_kernel_spmd

F32 = mybir.dt.float32
BF16 = mybir.dt.bfloat16
F8 = mybir.dt.float8e4
MAGIC = 12582912.0  # 1.5 * 2**23: (t + MAGIC) - MAGIC == round-half-even(t)
EPS = 1e-5

# Full problem shape (hardcoded per the task contract).
B, S, D_IN, D_OUT = 4, 2048, 4096, 16384
NCORES = 8
TOK = B * S  # 8192
O_SH = D_OUT // NCORES  # 2048 out-features per core


def split_multi_waits(nc):
    """The walrus build in this container encodes at most one sync-wait per
    instruction; Tile's sem-assigner can attach several.  Hoist the extras
    onto same-engine NoOps placed immediately before the instruction (engines
    execute their stream in order, so semantics are preserved)."""
    ctr = 0
    for f in nc.m.functions:
        for b in f.blocks:
            insts = b.instructions
            out = []
            changed = False
            for inst in insts:
                si = getattr(inst, "sync_info", None)
                waits = list(si.on_wait) if si is not None and si.on_wait else []
                if len(waits) > 1:
                    for wcond in waits[:-1]:
                        ctr += 1
                        nop = mybir.InstNoOp(
                            name=f"{inst.name}-wsplit{ctr}",
                            engine=inst.engine, ins=[], outs=[],
                            sync_info=mybir.SyncInfo(
                                on_wait=[wcond], on_update=[]),
                        )
                        nc.inst_map[nop.name] = nop
                        out.append(nop)
                    inst.sync_info = mybir.SyncInfo(
                        on_wait=[waits[-1]], on_update=list(si.on_update or []))
                    changed = True
                out.append(inst)
            if changed:
                b.instructions = out
    return ctr


def build_kernel(TOK=TOK, D=D_IN, O_SH=O_SH, NCORES=NCORES):
    """Emit the SPMD Bass program (identical on every core)."""
    P = 128
    assert TOK % (P * NCORES) == 0 and D % (2 * P) == 0 and O_SH % P == 0
    NTB = TOK // P            # token blocks
    NJ = D // P               # contraction (d) chunks of 128
    TSTAT = TOK // NCORES     # gamma stat-slice rows per core
    NXS = TSTAT // P          # stat tiles of [P, D]
    NFREE = min(512, O_SH)    # matmul moving free size
    NOC = (O_SH + NFREE - 1) // NFREE
    HD = D // 2               # x quant half width (per transpose)
    HJ = NJ // 2
    YF = min(1024, O_SH)      # y writeback chunk width
    NYC = O_SH // YF
    assert HD == O_SH, "junk tile for stats accum shares the q8 pool shape"
    w_elems = float(O_SH * NCORES) * float(D)

    nc = bass.Bass("TRN2", num_devices=NCORES)
    x = nc.dram_tensor("x", [TOK, D], F32, kind="ExternalInput").ap()
    xs = nc.dram_tensor("xs", [TSTAT, D], F32, kind="ExternalInput").ap()
    wt = nc.dram_tensor("wt", [D, O_SH], F32, kind="ExternalInput").ap()
    y = nc.dram_tensor("y", [TOK, O_SH], F32, kind="ExternalOutput").ap()

    RG = [list(range(NCORES))]
    AX = mybir.AxisListType.X
    OP = mybir.AluOpType
    ABS = mybir.ActivationFunctionType.Abs
    COPY = mybir.ActivationFunctionType.Copy

    with tile.TileContext(nc) as tc:
        with (
            tc.tile_pool(name="wqt", bufs=1) as wqt_pool,
            tc.tile_pool(name="sw", bufs=4) as sw,        # [P,O_SH] f32 stats+reread
            tc.tile_pool(name="rrp", bufs=4) as rrp,      # [P,O_SH] f32 early reread
            tc.tile_pool(name="big", bufs=3) as big,      # [P,D] f32 x rows
            tc.tile_pool(name="q8", bufs=1) as q8,        # [P,HD] bf16 xq halves
            tc.tile_pool(name="xqt", bufs=2) as xqt_pool, # [P,NJ,P] bf16
            tc.tile_pool(name="ysb", bufs=2) as ysb_pool, # [P,YF] f32
            tc.tile_pool(name="psum", bufs=2, space="PSUM") as psum_pool,
            tc.tile_pool(name="stats", bufs=1) as stats,
            tc.tile_pool(name="dram", bufs=1, space="DRAM") as dram,
        ):
            gparts = stats.tile([P, NXS], F32)
            bparts = stats.tile([P, NJ], F32)

            # ---- dummy collective: prepay the one-time CC warmup ----
            dmy = stats.tile([1, 8], F32)
            nc.vector.memset(dmy[:, :], 0.0)
            d_in = dram.tile([1, 8], F32)
            d_out = dram.tile([1, 8], F32)
            nc.scalar.dma_start(out=d_in[:, :], in_=dmy[:, :])
            nc.gpsimd.collective_compute(
                "AllReduce", OP.add, replica_groups=RG,
                ins=[d_in.opt()], outs=[d_out.opt()],
            )

            # ---- beta stats: wt shard split ~3:1 across the two DMA
            # queues (the scalar queue sustains only ~163GB/s on 8KB
            # packets); reduces split across the scalar engine
            # (activation Abs + accumulate) and the vector engine ----
            for j in range(NJ):
                wst = sw.tile([P, O_SH], F32, tag="sw")
                nc.sync.dma_start(
                    out=wst[:, :], in_=wt[j * P:(j + 1) * P, :])
                junk = q8.tile([P, O_SH], BF16, tag="q8")
                nc.scalar.activation(
                    junk[:, :], wst[:, :], ABS,
                    accum_out=bparts[:, j:j + 1],
                )

            # ---- wq re-read prefetch (sync queue, eager) ----
            rr_tiles = []

            RRNG = 4   # early re-reads in their own pool: no ring slot to
                       # wait on, so their descriptors never dam the queue

            def issue_rr(j):
                pool = rrp if j < RRNG else sw
                rrt = pool.tile([P, O_SH], F32,
                                tag="rrp" if j < RRNG else "sw")
                nc.sync.dma_start(
                    out=rrt[:, :], in_=wt[j * P:(j + 1) * P, :])
                rr_tiles.append(rrt)

            RRPRE = 8
            for j in range(min(RRNG, NJ)):
                issue_rr(j)

            # ---- gamma stats: xs slice on the sync queue; reduced on
            # vector right behind the wt reduces ----
            for t in range(NXS):
                xt = big.tile([P, D], F32, tag="big")
                nc.scalar.dma_start(out=xt[:, :], in_=xs[t * P:(t + 1) * P, :])
                nc.vector.tensor_reduce(
                    gparts[:, t:t + 1], xt[:, :], axis=AX, op=OP.max,
                    apply_absolute_value=True)
            gmax = stats.tile([P, 1], F32)
            nc.vector.tensor_reduce(
                gmax[:, :], gparts[:, :], axis=AX, op=OP.max)
            gm_d = dram.tile([1, P], F32)
            nc.scalar.dma_start(out=gm_d[0:1, :], in_=gmax[:, 0:1])
            gmrow = stats.tile([1, P], F32)
            nc.scalar.dma_start(out=gmrow[:, :], in_=gm_d[:, :])
            gmax_a = stats.tile([1, 8], F32)
            nc.vector.memset(gmax_a[:, :], 0.0)
            nc.vector.tensor_reduce(
                gmax_a[0:1, 0:1], gmrow[:, :], axis=AX, op=OP.max)
            g_in = dram.tile([1, 8], F32)
            g_out = dram.tile([1, 8], F32)
            nc.scalar.dma_start(out=g_in[:, :], in_=gmax_a[:, :])

            # beta partial: cross-partition fold via tiny DRAM round trip
            bsum = stats.tile([P, 1], F32)
            nc.vector.tensor_reduce(
                bsum[:, :], bparts[:, :], axis=AX, op=OP.add)
            bs_d = dram.tile([1, P], F32)
            nc.scalar.dma_start(out=bs_d[0:1, :], in_=bsum[:, 0:1])
            bsrow = stats.tile([1, P], F32)
            nc.scalar.dma_start(out=bsrow[:, :], in_=bs_d[:, :])
            bsum_a = stats.tile([1, 8], F32)
            nc.vector.memset(bsum_a[:, :], 0.0)
            nc.vector.tensor_reduce(
                bsum_a[0:1, 0:1], bsrow[:, :], axis=AX, op=OP.add)
            b_in = dram.tile([1, 8], F32)
            b_out = dram.tile([1, 8], F32)
            nc.scalar.dma_start(out=b_in[:, :], in_=bsum_a[:, :])

            # gamma AllReduce first (its input lands first), then beta;
            # each collective costs ~25-40us serially on gpsimd
            nc.gpsimd.collective_compute(
                "AllReduce", OP.max, replica_groups=RG,
                ins=[g_in.opt()], outs=[g_out.opt()],
            )
            nc.gpsimd.collective_compute(
                "AllReduce", OP.add, replica_groups=RG,
                ins=[b_in.opt()], outs=[b_out.opt()],
            )
            gallb = stats.tile([P, 8], F32)
            _gap = g_out.opt()
            nc.scalar.dma_start(
                out=gallb[:, :],
                in_=bass.AP(_gap.tensor, _gap.offset, [[0, P], [1, 8]]))
            ballb = stats.tile([P, 8], F32)
            _bap = b_out.opt()
            nc.scalar.dma_start(
                out=ballb[:, :],
                in_=bass.AP(_bap.tensor, _bap.offset, [[0, P], [1, 8]]))

            # ---- x token-block machinery ----
            xtiles = {}

            def load_x(tb, eng=None):
                xt = big.tile([P, D], F32, tag="big")
                (eng or nc.sync).dma_start(
                    out=xt[:, :], in_=x[tb * P:(tb + 1) * P, :])
                xtiles[tb] = xt

            gam_b = stats.tile([P, 1], F32)
            g7_b = stats.tile([P, 1], F32)
            s_bt = stats.tile([P, 1], F32)
            os1 = stats.tile([P, 1], F32)
            os_bt = stats.tile([P, 1], F32)
            bet_b = stats.tile([P, 1], F32)
            sclc = stats.tile([P, 1], F32)
            s_b = s_bt[:, 0:1]
            os_b = os_bt[:, 0:1]
            c_b = sclc[:, 0:1]   # beta/2

            def quant_x(tb, teng=None):
                """pass1 (fp32, in place) on gpsimd; pass2 (bf16 out) on
                vector (gpsimd's 16-bit output path is ~15x slower);
                transposes on teng's queue."""
                xf = xtiles.pop(tb)
                nc.vector.tensor_scalar(
                    xf[:, :], xf[:, :], s_b, MAGIC,
                    op0=OP.mult, op1=OP.add,
                )
                xqt = xqt_pool.tile([P, NJ, P], BF16, tag="xqt")
                for h in range(2):
                    xqh = q8.tile([P, HD], BF16, tag="q8")
                    nc.vector.tensor_scalar(
                        xqh[:, :], xf[:, h * HD:(h + 1) * HD], MAGIC, 127.0,
                        op0=OP.subtract, op1=OP.min,
                    )
                    (teng or nc.sync).dma_start_transpose(
                        out=xqt[:, h * HJ:(h + 1) * HJ, :], in_=xqh[:, :])
                return xqt

            # gamma-side derives, then the first four token-block quants —
            # all ahead of the beta-gated chase in the vector stream
            nc.vector.tensor_scalar_max(gam_b[:, :], gallb[:, 0:1], EPS)
            nc.vector.tensor_scalar_mul(g7_b[:, :], gam_b[:, :], 1.0 / 128.0)
            nc.vector.reciprocal(s_bt[:, :], g7_b[:, :])  # = 128/gamma

            load_x(0)
            load_x(1)
            load_x(2)
            xqts = {}
            xqts[0] = quant_x(0, teng=nc.scalar)
            xqts[1] = quant_x(1, teng=nc.scalar)
            load_x(3)
            xqts[2] = quant_x(2, teng=nc.scalar)
            xqts[3] = quant_x(3, teng=nc.scalar)

            # deferred follow-on re-reads (cannot ride with the early
            # prefetch: their ring slots free only as the stats reduces and
            # the chase drain, and a blocked descriptor stalls everything
            # behind it on its engine)
            for j in range(min(RRNG, NJ), min(2 * RRPRE, NJ)):
                issue_rr(j)

            # ---- beta-side derives (vector; wait on the beta AR) ----
            nc.vector.tensor_scalar(
                bet_b[:, :], ballb[:, 0:1], 1.0 / w_elems, EPS,
                op0=OP.mult, op1=OP.max,
            )
            # c = 0.5*max(sum/N, eps) == max(sum/(2N), eps/2) exactly
            nc.vector.tensor_scalar(
                sclc[:, :], ballb[:, 0:1], 0.5 / w_elems, 0.5 * EPS,
                op0=OP.mult, op1=OP.max,
            )
            nc.vector.tensor_tensor(
                os1[:, :], bet_b[:, :], gam_b[:, :], op=OP.mult)
            nc.vector.tensor_scalar_mul(os_bt[:, :], os1[:, :], 1.0 / 128.0)

            # ---- the wq quantize chase ----
            wqts = []
            for j in range(NJ):
                wqj = wqt_pool.tile([P, O_SH], F8, tag=f"wq{j}")
                # |w| in place on the scalar engine, then the compare
                nc.scalar.activation(
                    rr_tiles[j][:, :], rr_tiles[j][:, :], ABS)
                nc.vector.tensor_scalar(
                    wqj[:, :], rr_tiles[j][:, :], c_b, None,
                    op0=OP.is_gt,
                )
                wqts.append(wqj)
                if 2 * RRPRE <= j + RRPRE < NJ:
                    issue_rr(j + RRPRE)

            # ---- matmul phase ----
            def emit_scale_out(tb, pt):
                for oc in range(NYC):
                    yt = ysb_pool.tile([P, YF], F32, tag="ysb")
                    nc.scalar.activation(
                        yt[:, :], pt[:, oc * YF:(oc + 1) * YF],
                        COPY, bias=0.0, scale=os_b,
                    )
                    nc.sync.dma_start(
                        out=y[tb * P:(tb + 1) * P, oc * YF:(oc + 1) * YF],
                        in_=yt[:, :])

            def emit_prefetch(tb):
                if tb < NTB:
                    load_x(tb)
                    xqts[tb] = quant_x(tb)

            def emit_matmuls(pt, xqt):
                for j in range(NJ):
                    for oc in range(NOC):
                        nc.tensor.matmul(
                            pt[:, oc * NFREE:(oc + 1) * NFREE],
                            xqt[:, j:j + 1, :],
                            wqts[j][:, oc * NFREE:(oc + 1) * NFREE],
                            start=(j == 0), stop=(j == NJ - 1),
                        )

            # first two token blocks interleaved per-j: two PSUM groups in
            # flight so the PE drains each wq slab as it is quantized
            pt0 = psum_pool.tile([P, O_SH], F32, tag="pt")
            pt1 = psum_pool.tile([P, O_SH], F32, tag="pt")
            xq0, xq1 = xqts.pop(0), xqts.pop(1)
            for j in range(NJ):
                for pt, xqt in ((pt0, xq0), (pt1, xq1)):
                    for oc in range(NOC):
                        nc.tensor.matmul(
                            pt[:, oc * NFREE:(oc + 1) * NFREE],
                            xqt[:, j:j + 1, :],
                            wqts[j][:, oc * NFREE:(oc + 1) * NFREE],
                            start=(j == 0), stop=(j == NJ - 1),
                        )
            emit_scale_out(0, pt0)
            emit_scale_out(1, pt1)
            emit_prefetch(4)
            emit_prefetch(5)

            for tb in range(2, NTB):
                xqt = xqts.pop(tb)
                pt = psum_pool.tile([P, O_SH], F32, tag="pt")
                emit_matmuls(pt, xqt)
                emit_scale_out(tb, pt)
                emit_prefetch(tb + 4)

    split_multi_waits(nc)
    return nc


_CACHE = {}


def _get_nc(key, **kw):
    if key not in _CACHE:
        _CACHE[key] = build_kernel(**kw)
    return _CACHE[key]


def make_in_maps(x2d, w2d, ncores=NCORES):
    tok = x2d.shape[0]
    tstat = tok // ncores
    osh = w2d.shape[0] // ncores
    wt_full = np.ascontiguousarray(w2d.T)
    in_maps = []
    for c in range(ncores):
        in_maps.append({
            "x": x2d,
            "xs": x2d[c * tstat:(c + 1) * tstat],
            "wt": np.ascontiguousarray(wt_full[:, c * osh:(c + 1) * osh]),
        })
    return in_maps


def kernel(x, weight, _trace=False, _tmpdir=None):
    assert x.shape == (B, S, D_IN) and weight.shape == (D_OUT, D_IN)
    x2d = np.ascontiguousarray(x.reshape(TOK, D_IN), dtype=np.float32)
    w2d = np.ascontiguousarray(weight, dtype=np.float32)
    nc = _get_nc("full")
    res = run_bass_kernel_spmd(
        nc, make_in_maps(x2d, w2d), core_ids=list(range(NCORES)),
        trace=_trace, tmpdir=_tmpdir,
    )
    y = np.concatenate([r["y"] for r in res.results], axis=1)
    out = y.reshape(B, S, D_OUT)
    if _trace:
        return out, res
    return out
